# revision 1
# baseline (speedup 1.0000x reference)
"""MinGRU layer kernel for Trainium2 (8 NeuronCores, data-parallel over batch).

Math per batch element b (reference semantics):
    z_t = Wz @ x_t + bz ; g_t = sigmoid(z_t)
    u_t = Wh @ x_t + bh
    h_t = (1-g_t) * h_{t-1} + g_t * u_t     (linear recurrence along T)
    y_t = Wo @ h_t + bo

Device layout: hidden dim on partitions (8 tiles x 128), time on the free
dim, chunked by TC=512 columns. The recurrence runs on the DVE
``tensor_tensor_scan`` instruction (state = a*state + b along the free dim)
with a = sigmoid(-z-bz) = 1-g and b = (u+bh)*g. Matmuls take bf16 inputs
with fp32 PSUM accumulation; scan inputs stay fp32; h is stored bf16 for
the output matmul. Output-chunk matmuls are deferred one chunk so the PE
never waits on the serial scan chain.

Sharding: batch B=8 -> one batch element per core; weights broadcast.
"""

import numpy as np
import ml_dtypes

import concourse.bass as bass
import concourse.bacc as bacc
import concourse.mybir as mybir
import concourse.tile as tile
from concourse.bass_utils import run_bass_kernel_spmd
from concourse.bass_interp import get_hw_module
from concourse.tile_rust import add_dep_helper

B, T, I, H, O = 8, 4096, 1024, 1024, 1024
P = 128
TC = 512  # time chunk (matmul free dim / PSUM bank)

BF16 = mybir.dt.bfloat16
F32 = mybir.dt.float32
NPBF16 = ml_dtypes.bfloat16

AL = mybir.AluOpType
AF = mybir.ActivationFunctionType


def build_program(t=T, i=I, h=H, o=O, tc_len=TC, n_cores=8, enable_asserts=False):
    KI, MH, MO, NCH = i // P, h // P, o // P, t // tc_len
    nc = bacc.Bacc(
        "TRN2",
        target_bir_lowering=False,
        debug=False,
        enable_asserts=enable_asserts,
        num_devices=n_cores,
    )

    # Host pre-tiled layouts (see kernel() below for the exact packing).
    xT = nc.dram_tensor("xT", [P, KI, t], BF16, kind="ExternalInput")
    wz = nc.dram_tensor("wz", [P, MH, KI, P], BF16, kind="ExternalInput")
    wh = nc.dram_tensor("wh", [P, MH, KI, P], BF16, kind="ExternalInput")
    wo = nc.dram_tensor("wo", [P, MO, MH, P], BF16, kind="ExternalInput")
    bzd = nc.dram_tensor("bz", [P, MH], F32, kind="ExternalInput")
    nbzd = nc.dram_tensor("nbz", [P, MH], F32, kind="ExternalInput")
    bhd = nc.dram_tensor("bh", [P, MH], F32, kind="ExternalInput")
    bod = nc.dram_tensor("bo", [P, MO], F32, kind="ExternalInput")
    out = nc.dram_tensor("out", [P, MO, t], F32, kind="ExternalOutput")

    with tile.TileContext(nc, pool_alloc_mode="queue") as tcx:
        with (
            tcx.tile_pool(name="weights", bufs=1) as wpool,
            tcx.tile_pool(name="xin", bufs=2) as xpool,
            tcx.tile_pool(name="gtmp", bufs=4) as gpool,
            tcx.tile_pool(name="ab", bufs=2) as abpool,
            tcx.tile_pool(name="hsb", bufs=3) as hpool,
            tcx.tile_pool(name="osb", bufs=8) as opool,
            tcx.tile_pool(name="zups", bufs=5, space=bass.MemorySpace.PSUM) as zups,
            tcx.tile_pool(name="ops", bufs=3, space=bass.MemorySpace.PSUM) as ops,
        ):
            # Startup critical path: the first z matmul group needs only
            # x0 + wz[m0]. Those (plus biases and the m=1 weights) are
            # loaded first, split across the sync and scalar DGE streams;
            # all remaining weight slices are gated behind the first
            # matmul (see below) so the critical transfers get the HBM
            # pipe to themselves.
            x_first = xpool.tile([P, KI, tc_len], BF16, tag="x")
            wz_s = wpool.tile([P, MH, KI, P], BF16, tag="wz")
            wh_s = wpool.tile([P, MH, KI, P], BF16, tag="wh")
            wo_s = wpool.tile([P, MO, MH, P], BF16, tag="wo")

            nc.sync.dma_start(x_first[:], xT[:, :, 0:tc_len])
            nc.scalar.dma_start(wz_s[:, 0], wz[:, 0])
            nc.scalar.dma_start(wh_s[:, 0], wh[:, 0])
            if MH > 1:
                nc.sync.dma_start(wz_s[:, 1], wz[:, 1])
                nc.sync.dma_start(wh_s[:, 1], wh[:, 1])

            bz_s = wpool.tile([P, MH], F32, tag="bz")
            nc.gpsimd.dma_start(bz_s[:], bzd[:])
            nbz_s = wpool.tile([P, MH], F32, tag="nbz")
            nc.gpsimd.dma_start(nbz_s[:], nbzd[:])
            bh_s = wpool.tile([P, MH], F32, tag="bh")
            nc.gpsimd.dma_start(bh_s[:], bhd[:])
            bo_s = wpool.tile([P, MO], F32, tag="bo")
            nc.gpsimd.dma_start(bo_s[:], bod[:])


            def emit_out_chunk(c, h_tile):
                sl = slice(c * tc_len, (c + 1) * tc_len)
                for mo in range(MO):
                    o_ps = ops.tile([P, tc_len], F32, tag="o")
                    for k in range(MH):
                        nc.tensor.matmul(
                            o_ps[:],
                            wo_s[:, mo, k, :],
                            h_tile[:, k, :],
                            start=(k == 0),
                            stop=(k == MH - 1),
                        )
                    o_sb = opool.tile([P, tc_len], F32, tag="osb")
                    # Bias-add on the scalar engine: keeps o-PSUM recycling
                    # off the DVE queue (which carries the scan chain).
                    nc.scalar.activation(
                        o_sb[:], o_ps[:], AF.Identity, bias=bo_s[:, mo : mo + 1]
                    )
                    nc.sync.dma_start(out[:, mo, sl], o_sb[:])

            h_prev = None
            for c in range(NCH):
                sl = slice(c * tc_len, (c + 1) * tc_len)
                # x chunks ride the gpsimd DGE stream so they don't queue
                # behind the weight loads on the sync engine's stream.
                if c == 0:
                    x_s = x_first
                else:
                    x_s = xpool.tile([P, KI, tc_len], BF16, tag="x")
                    nc.gpsimd.dma_start(x_s[:], xT[:, :, sl])

                a_s = abpool.tile([P, MH, tc_len], F32, tag="a")
                b_s = abpool.tile([P, MH, tc_len], F32, tag="b")
                h_s = hpool.tile([P, MH, tc_len], BF16, tag="h")

                for m in range(MH):
                    z_ps = zups.tile([P, tc_len], F32, tag="zu")
                    for k in range(KI):
                        mm = nc.tensor.matmul(
                            z_ps[:],
                            wz_s[:, m, k, :],
                            x_s[:, k, :],
                            start=(k == 0),
                            stop=(k == KI - 1),
                        )
                        if c == 0 and m == 0 and k == 0:
                            # Bulk weight loads wait for the first matmul so
                            # its critical inputs (x0, wz/wh m0-m1) get the
                            # HBM pipe to themselves during startup.
                            for mw in range(2, MH):
                                for w_s, w_d in ((wz_s, wz), (wh_s, wh)):
                                    d = nc.sync.dma_start(w_s[:, mw], w_d[:, mw])
                                    add_dep_helper(
                                        d.ins, mm.ins, True, "bulk weights after start"
                                    )
                            for mo_ in range(MO):
                                d = nc.sync.dma_start(wo_s[:, mo_], wo[:, mo_])
                                add_dep_helper(
                                    d.ins, mm.ins, True, "bulk weights after start"
                                )
                    u_ps = zups.tile([P, tc_len], F32, tag="zu")
                    for k in range(KI):
                        nc.tensor.matmul(
                            u_ps[:],
                            wh_s[:, m, k, :],
                            x_s[:, k, :],
                            start=(k == 0),
                            stop=(k == KI - 1),
                        )
                    g_s = gpool.tile([P, tc_len], F32, tag="g")
                    # g = sigmoid(z + bz)
                    nc.scalar.activation(
                        g_s[:], z_ps[:], AF.Sigmoid, bias=bz_s[:, m : m + 1], scale=1.0
                    )
                    # a = 1 - g = sigmoid(-z - bz)
                    nc.scalar.activation(
                        a_s[:, m, :],
                        z_ps[:],
                        AF.Sigmoid,
                        bias=nbz_s[:, m : m + 1],
                        scale=-1.0,
                    )
                    # b = (u + bh) * g
                    nc.vector.scalar_tensor_tensor(
                        b_s[:, m, :], u_ps[:], bh_s[:, m : m + 1], g_s[:], AL.add, AL.mult
                    )
                    # h[:, t] = a[:, t] * h[:, t-1] + b[:, t]
                    init = 0.0 if c == 0 else h_prev[:, m, tc_len - 1 : tc_len]
                    nc.vector.tensor_tensor_scan(
                        h_s[:, m, :], a_s[:, m, :], b_s[:, m, :], init, AL.mult, AL.add
                    )

                # Output matmuls for the previous chunk, emitted after this
                # chunk's gate/update matmuls so the PE stream never has to
                # wait on the (serial) scan chain.
                if c > 0:
                    emit_out_chunk(c - 1, h_prev)
                h_prev = h_s
            emit_out_chunk(NCH - 1, h_prev)

    nc.compile()
    return nc


_CACHED_NC = None


def _get_nc():
    global _CACHED_NC
    if _CACHED_NC is None:
        _CACHED_NC = build_program()
    return _CACHED_NC


# Set by test harnesses that want a profile: kernel() stores the raw
# BassKernelResults of the last run here when TRACE is truthy.
TRACE = False
LAST_RESULTS = None


def _pack_weight(w):
    # [out_dim, in_dim] -> lhsT tiles [P, M_tiles, K_tiles, P] where
    # arr[p, m, k, q] = w[m*P + q, k*P + p]
    kd, md = w.shape[1] // P, w.shape[0] // P
    return np.ascontiguousarray(
        w.T.reshape(kd, P, md, P).transpose(1, 2, 0, 3).astype(NPBF16)
    )


def kernel(**inputs):
    global LAST_RESULTS
    xs = np.asarray(inputs["xs"], np.float32)
    Wz = np.asarray(inputs["Wz"], np.float32)
    bz = np.asarray(inputs["bz"], np.float32)
    Wh = np.asarray(inputs["Wh"], np.float32)
    bh = np.asarray(inputs["bh"], np.float32)
    Wo = np.asarray(inputs["Wo"], np.float32)
    bo = np.asarray(inputs["bo"], np.float32)

    KI, MH, MO = I // P, H // P, O // P

    wz_t = _pack_weight(Wz)
    wh_t = _pack_weight(Wh)
    wo_t = _pack_weight(Wo)
    bz_p = np.ascontiguousarray(bz.reshape(MH, P).T)
    nbz_p = np.ascontiguousarray((-bz).reshape(MH, P).T)
    bh_p = np.ascontiguousarray(bh.reshape(MH, P).T)
    bo_p = np.ascontiguousarray(bo.reshape(MO, P).T)

    in_maps = []
    for b in range(B):
        # [T, I] -> [I, T] bf16 -> [P, KI, T] with x[p, k, t] = xs[b, t, k*P+p]
        xb = xs[b].T.astype(NPBF16)
        xb = np.ascontiguousarray(xb.reshape(KI, P, T).transpose(1, 0, 2))
        in_maps.append(
            {
                "xT": xb,
                "wz": wz_t,
                "wh": wh_t,
                "wo": wo_t,
                "bz": bz_p,
                "nbz": nbz_p,
                "bh": bh_p,
                "bo": bo_p,
            }
        )

    nc = _get_nc()
    old_m = nc.m
    nc.m = get_hw_module(nc.m)
    try:
        res = run_bass_kernel_spmd(
            nc, in_maps, core_ids=list(range(B)), trace=bool(TRACE)
        )
    finally:
        nc.m = old_m
    LAST_RESULTS = res

    out_full = np.empty((B, T, O), np.float32)
    for b in range(B):
        # [P, MO, T] -> [O, T] -> [T, O]
        ob = res.results[b]["out"]
        out_full[b] = ob.transpose(1, 0, 2).reshape(O, T).T
    return out_full



# revision 4
# speedup vs baseline: 1.1236x; 1.1236x over previous
"""MinGRU layer kernel for Trainium2 (8 NeuronCores, data-parallel over batch).

Math per batch element b (reference semantics):
    z_t = Wz @ x_t + bz ; g_t = sigmoid(z_t)
    u_t = Wh @ x_t + bh
    h_t = (1-g_t) * h_{t-1} + g_t * u_t     (linear recurrence along T)
    y_t = Wo @ h_t + bo

Device layout: hidden dim on partitions (8 tiles x 128), time on the free
dim, chunked by a variable schedule (256,256,512x6,256,128,128). The
recurrence runs on the DVE ``tensor_tensor_scan`` instruction (state =
a*state + b along the free dim) with a = sigmoid(-z-bz) = 1-g and
b = (u+bh)*g. Matmuls take bf16 inputs with fp32 PSUM accumulation; scan
inputs stay fp32; h is stored bf16 for the output matmul. Output-chunk
matmuls are deferred one chunk so the PE never waits on the serial scan
chain; the small trailing chunks shrink the non-overlapped tail
(final out-GEMM + bias + store).

Startup: the PE is fed as data trickles in. x chunk 0 is loaded as per-k
slices on the sync DGE stream; weights ride the scalar DGE stream in
exact consumption order (m0 per-k, m1, m2..m7, wo) so the first matmul
only waits for ~96KB. A burst of dummy matmuls on zeroed SBUF warms the
PE_HAM clock gate (1.2 -> 2.4 GHz) during the DMA wait so real matmuls
start at full clock.

Sharding: batch B=8 -> one batch element per core; weights broadcast.
"""

import numpy as np
import ml_dtypes

import concourse.bass as bass
import concourse.bacc as bacc
import concourse.mybir as mybir
import concourse.tile as tile
from concourse.bass_utils import run_bass_kernel_spmd
from concourse.bass_interp import get_hw_module

B, T, I, H, O = 8, 4096, 1024, 1024, 1024
P = 128
TCMAX = 512  # max time chunk (PSUM bank / matmul free dim)
# Chunk schedule: small head chunk (fast start while DMA streams), large
# middle chunks (min per-instruction overhead), small tail chunks (small
# non-overlapped final out-GEMM + store).
CHUNKS = [256, 256, 512, 512, 512, 512, 512, 512, 256, 128, 128]
assert sum(CHUNKS) == T
N_WARM = 8  # dummy matmuls to warm the PE_HAM clock gate during startup

BF16 = mybir.dt.bfloat16
F32 = mybir.dt.float32
NPBF16 = ml_dtypes.bfloat16

AL = mybir.AluOpType
AF = mybir.ActivationFunctionType


def build_program(t=T, i=I, h=H, o=O, chunks=None, n_cores=8, enable_asserts=False):
    KI, MH, MO = i // P, h // P, o // P
    if chunks is None:
        chunks = CHUNKS if t == T else [min(TCMAX, t)] * (t // min(TCMAX, t))
    assert sum(chunks) == t
    nc = bacc.Bacc(
        "TRN2",
        target_bir_lowering=False,
        debug=False,
        enable_asserts=enable_asserts,
        num_devices=n_cores,
    )

    # Host pre-tiled layouts (see kernel() below for the exact packing).
    xT = nc.dram_tensor("xT", [P, KI, t], BF16, kind="ExternalInput")
    wz = nc.dram_tensor("wz", [P, MH, KI, P], BF16, kind="ExternalInput")
    wh = nc.dram_tensor("wh", [P, MH, KI, P], BF16, kind="ExternalInput")
    wo = nc.dram_tensor("wo", [P, MO, MH, P], BF16, kind="ExternalInput")
    bzd = nc.dram_tensor("bz", [P, MH], F32, kind="ExternalInput")
    nbzd = nc.dram_tensor("nbz", [P, MH], F32, kind="ExternalInput")
    bhd = nc.dram_tensor("bh", [P, MH], F32, kind="ExternalInput")
    bod = nc.dram_tensor("bo", [P, MO], F32, kind="ExternalInput")
    out = nc.dram_tensor("out", [P, MO, t], F32, kind="ExternalOutput")

    with tile.TileContext(nc, pool_alloc_mode="queue") as tcx:
        with (
            tcx.tile_pool(name="weights", bufs=1) as wpool,
            tcx.tile_pool(name="xin", bufs=2) as xpool,
            tcx.tile_pool(name="gtmp", bufs=4) as gpool,
            tcx.tile_pool(name="ab", bufs=2) as abpool,
            tcx.tile_pool(name="hsb", bufs=3) as hpool,
            tcx.tile_pool(name="osb", bufs=8) as opool,
            tcx.tile_pool(name="zups", bufs=5, space=bass.MemorySpace.PSUM) as zups,
            tcx.tile_pool(name="ops", bufs=3, space=bass.MemorySpace.PSUM) as ops,
        ):
            c0 = chunks[0]
            x_first = xpool.tile([P, KI, TCMAX], BF16, tag="x")
            wz_s = wpool.tile([P, MH, KI, P], BF16, tag="wz")
            wh_s = wpool.tile([P, MH, KI, P], BF16, tag="wh")
            wo_s = wpool.tile([P, MO, MH, P], BF16, tag="wo")

            # PE warm-up: dummy matmuls on zeroed SBUF into a scratch PSUM
            # bank. They have no DMA dependencies, so they run as soon as
            # the engines boot and hold the PE_HAM activity window busy
            # while the startup DMAs stream; real matmuls then start at
            # 2.4 GHz instead of 1.2 GHz. The scratch PSUM buffer comes
            # from the out pool, which has no real use until the first
            # out-chunk emission (~30us in), long after the dummies retire.
            dummy_w = wpool.tile([P, P], BF16, tag="dummyw")
            dummy_x = wpool.tile([P, TCMAX], BF16, tag="dummyx")
            nc.gpsimd.memset(dummy_w[:], 0)
            nc.gpsimd.memset(dummy_x[:], 0)
            warm_ps = ops.tile([P, TCMAX], F32, tag="o")
            for _ in range(N_WARM):
                nc.tensor.matmul(
                    warm_ps[:], dummy_w[:], dummy_x[:], start=True, stop=True
                )

            # Startup loads, ordered by consumption. sync stream: x chunk 0
            # per-k slices (the first matmul needs only k=0). scalar
            # stream: weights in exact consumption order — per-k m0 slices
            # (z group m0 starts after 32KB), then m1, m2..m7, then wo.
            # Per-engine DGE streams process their descriptors in order,
            # each striping across all 16 HW queues, so this ordering IS
            # the priority scheme; no explicit gating needed.
            for k in range(KI):
                nc.sync.dma_start(x_first[:, k, 0:c0], xT[:, k, 0:c0])
            for k in range(KI):
                nc.scalar.dma_start(wz_s[:, 0, k], wz[:, 0, k])
            for k in range(KI):
                nc.scalar.dma_start(wh_s[:, 0, k], wh[:, 0, k])
            if MH > 1:
                nc.scalar.dma_start(wz_s[:, 1], wz[:, 1])
                nc.scalar.dma_start(wh_s[:, 1], wh[:, 1])
            for mw in range(2, MH):
                nc.scalar.dma_start(wz_s[:, mw], wz[:, mw])
                nc.scalar.dma_start(wh_s[:, mw], wh[:, mw])
            for mo_ in range(MO):
                nc.scalar.dma_start(wo_s[:, mo_], wo[:, mo_])

            bz_s = wpool.tile([P, MH], F32, tag="bz")
            nc.gpsimd.dma_start(bz_s[:], bzd[:])
            nbz_s = wpool.tile([P, MH], F32, tag="nbz")
            nc.gpsimd.dma_start(nbz_s[:], nbzd[:])
            bh_s = wpool.tile([P, MH], F32, tag="bh")
            nc.gpsimd.dma_start(bh_s[:], bhd[:])
            bo_s = wpool.tile([P, MO], F32, tag="bo")
            nc.gpsimd.dma_start(bo_s[:], bod[:])

            starts = [sum(chunks[:c]) for c in range(len(chunks))]

            def emit_out_chunk(c, h_tile):
                ln = chunks[c]
                sl = slice(starts[c], starts[c] + ln)
                for mo in range(MO):
                    o_ps = ops.tile([P, TCMAX], F32, tag="o")
                    for k in range(MH):
                        nc.tensor.matmul(
                            o_ps[:, 0:ln],
                            wo_s[:, mo, k, :],
                            h_tile[:, k, 0:ln],
                            start=(k == 0),
                            stop=(k == MH - 1),
                        )
                    o_sb = opool.tile([P, TCMAX], F32, tag="osb")
                    # Bias-add on the scalar engine: keeps o-PSUM recycling
                    # off the DVE queue (which carries the scan chain).
                    nc.scalar.activation(
                        o_sb[:, 0:ln],
                        o_ps[:, 0:ln],
                        AF.Identity,
                        bias=bo_s[:, mo : mo + 1],
                    )
                    nc.sync.dma_start(out[:, mo, sl], o_sb[:, 0:ln])

            h_prev = None
            ln_prev = None
            for c, ln in enumerate(chunks):
                sl = slice(starts[c], starts[c] + ln)
                # x chunks ride the gpsimd DGE stream so they don't queue
                # behind the weight loads on the scalar stream; the x pool's
                # double buffering (WAR deps) keeps chunk c+2 from competing
                # with startup-critical transfers.
                if c == 0:
                    x_s = x_first
                else:
                    x_s = xpool.tile([P, KI, TCMAX], BF16, tag="x")
                    nc.gpsimd.dma_start(x_s[:, :, 0:ln], xT[:, :, sl])

                a_s = abpool.tile([P, MH, TCMAX], F32, tag="a")
                b_s = abpool.tile([P, MH, TCMAX], F32, tag="b")
                h_s = hpool.tile([P, MH, TCMAX], BF16, tag="h")

                for m in range(MH):
                    z_ps = zups.tile([P, TCMAX], F32, tag="zu")
                    for k in range(KI):
                        nc.tensor.matmul(
                            z_ps[:, 0:ln],
                            wz_s[:, m, k, :],
                            x_s[:, k, 0:ln],
                            start=(k == 0),
                            stop=(k == KI - 1),
                        )
                    u_ps = zups.tile([P, TCMAX], F32, tag="zu")
                    for k in range(KI):
                        nc.tensor.matmul(
                            u_ps[:, 0:ln],
                            wh_s[:, m, k, :],
                            x_s[:, k, 0:ln],
                            start=(k == 0),
                            stop=(k == KI - 1),
                        )
                    g_s = gpool.tile([P, TCMAX], F32, tag="g")
                    # g = sigmoid(z + bz)
                    nc.scalar.activation(
                        g_s[:, 0:ln],
                        z_ps[:, 0:ln],
                        AF.Sigmoid,
                        bias=bz_s[:, m : m + 1],
                        scale=1.0,
                    )
                    # a = 1 - g = sigmoid(-z - bz)
                    nc.scalar.activation(
                        a_s[:, m, 0:ln],
                        z_ps[:, 0:ln],
                        AF.Sigmoid,
                        bias=nbz_s[:, m : m + 1],
                        scale=-1.0,
                    )
                    # b = (u + bh) * g
                    nc.vector.scalar_tensor_tensor(
                        b_s[:, m, 0:ln],
                        u_ps[:, 0:ln],
                        bh_s[:, m : m + 1],
                        g_s[:, 0:ln],
                        AL.add,
                        AL.mult,
                    )
                    # h[:, t] = a[:, t] * h[:, t-1] + b[:, t]
                    init = 0.0 if c == 0 else h_prev[:, m, ln_prev - 1 : ln_prev]
                    nc.vector.tensor_tensor_scan(
                        h_s[:, m, 0:ln],
                        a_s[:, m, 0:ln],
                        b_s[:, m, 0:ln],
                        init,
                        AL.mult,
                        AL.add,
                    )

                # Output matmuls for the previous chunk, emitted after this
                # chunk's gate/update matmuls so the PE stream never has to
                # wait on the (serial) scan chain.
                if c > 0:
                    emit_out_chunk(c - 1, h_prev)
                h_prev = h_s
                ln_prev = ln
            emit_out_chunk(len(chunks) - 1, h_prev)

    nc.compile()
    return nc


_CACHED_NC = None


def _get_nc():
    global _CACHED_NC
    if _CACHED_NC is None:
        _CACHED_NC = build_program()
    return _CACHED_NC


# Set by test harnesses that want a profile: kernel() stores the raw
# BassKernelResults of the last run here when TRACE is truthy.
TRACE = False
LAST_RESULTS = None


def _pack_weight(w):
    # [out_dim, in_dim] -> lhsT tiles [P, M_tiles, K_tiles, P] where
    # arr[p, m, k, q] = w[m*P + q, k*P + p]
    kd, md = w.shape[1] // P, w.shape[0] // P
    return np.ascontiguousarray(
        w.T.reshape(kd, P, md, P).transpose(1, 2, 0, 3).astype(NPBF16)
    )


def kernel(**inputs):
    global LAST_RESULTS
    xs = np.asarray(inputs["xs"], np.float32)
    Wz = np.asarray(inputs["Wz"], np.float32)
    bz = np.asarray(inputs["bz"], np.float32)
    Wh = np.asarray(inputs["Wh"], np.float32)
    bh = np.asarray(inputs["bh"], np.float32)
    Wo = np.asarray(inputs["Wo"], np.float32)
    bo = np.asarray(inputs["bo"], np.float32)

    KI, MH, MO = I // P, H // P, O // P

    wz_t = _pack_weight(Wz)
    wh_t = _pack_weight(Wh)
    wo_t = _pack_weight(Wo)
    bz_p = np.ascontiguousarray(bz.reshape(MH, P).T)
    nbz_p = np.ascontiguousarray((-bz).reshape(MH, P).T)
    bh_p = np.ascontiguousarray(bh.reshape(MH, P).T)
    bo_p = np.ascontiguousarray(bo.reshape(MO, P).T)

    in_maps = []
    for b in range(B):
        # [T, I] -> [I, T] bf16 -> [P, KI, T] with x[p, k, t] = xs[b, t, k*P+p]
        xb = xs[b].T.astype(NPBF16)
        xb = np.ascontiguousarray(xb.reshape(KI, P, T).transpose(1, 0, 2))
        in_maps.append(
            {
                "xT": xb,
                "wz": wz_t,
                "wh": wh_t,
                "wo": wo_t,
                "bz": bz_p,
                "nbz": nbz_p,
                "bh": bh_p,
                "bo": bo_p,
            }
        )

    nc = _get_nc()
    old_m = nc.m
    nc.m = get_hw_module(nc.m)
    try:
        res = run_bass_kernel_spmd(
            nc, in_maps, core_ids=list(range(B)), trace=bool(TRACE)
        )
    finally:
        nc.m = old_m
    LAST_RESULTS = res

    out_full = np.empty((B, T, O), np.float32)
    for b in range(B):
        # [P, MO, T] -> [O, T] -> [T, O]
        ob = res.results[b]["out"]
        out_full[b] = ob.transpose(1, 0, 2).reshape(O, T).T
    return out_full
